# revision 2
# baseline (speedup 1.0000x reference)
"""Trainium2 Bass kernel for nn_DeformConv2d_69621419868390.

With zero offsets the deformable sampling degenerates to integer sampling:
    out[b, c, 3*i+kx, 3*j+ky] = s * xp[b, c, i+kx, j+ky]
where xp is the reflection-padded (258x258) input plane and s doubles on the
padded-boundary row/col 257 (x4 at the corner).

Equivalently, defining per input row r the expanded row
    F(r)[0]      = x[r, 1]
    F(r)[3j+1]   = x[r, j]        j in [0, 256)
    F(r)[3j+2]   = x[r, j+1]      j in [0, 255)
    F(r)[767]    = 2 * x[r, 254]
    F(r)[3j+3]   = x[r, j]        j in [0, 255)
the output plane rows are:
    out[3i+1] = F(i)              i in [0, 256)
    out[3i+3] = F(i)              i in [0, 255)
    out[3i-1] = F(i)              i in [1, 256)
    out[0]    = F(1)
    out[767]  = 2 * F(254)

Pure data parallel: 4*32 = 128 independent planes, 16 per NeuronCore.
Per core: load plane -> DVE strided copies expand rows in SBUF -> strided
HWDGE DMA stores scatter the 768-row output plane.
"""

import numpy as np

N_CORES = 8
PLANES_PER_CORE = 16
H = 256
W = 256

_NC_CACHE = {}


def _build_nc(n_iter: int = 1):
    import concourse.bacc as bacc
    import concourse.mybir as mybir
    from concourse.tile import TileContext

    F32 = mybir.dt.float32

    nc = bacc.Bacc(
        "TRN2", target_bir_lowering=False, debug=False, num_devices=N_CORES
    )
    x = nc.dram_tensor(
        "x", [PLANES_PER_CORE, H, W], F32, kind="ExternalInput"
    )
    y = nc.dram_tensor(
        "y", [PLANES_PER_CORE, 3 * H, 3 * W], F32, kind="ExternalOutput"
    )
    xa = x.ap()
    ya = y.ap()

    with TileContext(nc) as tc:
        with tc.tile_pool(name="io", bufs=4) as pool:
            for _ in range(n_iter):
                for p in range(PLANES_PER_CORE):
                    _build_plane(nc, pool, xa, ya, p, F32)
    nc.compile()
    return nc


def _build_plane(nc, pool, xa, ya, p, F32):
    # I: partition q holds input rows (2q, 2q+1); I2[q, k, c] = x[2q+k, c]
    I = pool.tile([128, 2 * W], F32, tag="I")
    # O: partition q holds F(2q) | F(2q+1); O2[q, k, j] = F(2q+k)[j]
    O = pool.tile([128, 2 * 768], F32, tag="O")
    # G: partition 127 holds 2*F(254) for the final output row
    G = pool.tile([128, 768], F32, tag="G")

    I2 = I.rearrange("q (k c) -> q k c", c=W)
    O2 = O.rearrange("q (k c) -> q k c", c=768)

    src = xa[p].rearrange("(q r) c -> q (r c)", r=2)  # [128, 512] contiguous
    nc.sync.dma_start(I[:, :], src)

    # Row expansion: three interleaved strided copies + two edge columns.
    nc.vector.tensor_copy(O2[:, :, 1:767:3], I2[:, :, 0:256])   # F[3j+1]=x[j]
    nc.vector.tensor_copy(O2[:, :, 2:765:3], I2[:, :, 1:256])   # F[3j+2]=x[j+1]
    nc.vector.tensor_copy(O2[:, :, 3:766:3], I2[:, :, 0:255])   # F[3j+3]=x[j]
    nc.vector.tensor_copy(O2[:, :, 0:1], I2[:, :, 1:2])         # F[0]=x[1]
    nc.scalar.mul(O2[:, :, 767:768], I2[:, :, 254:255], 2.0)    # F[767]=2x[254]
    # G row: 2 * F(254) (also yields the x4 corner from F[767]).
    # Compute engines need a partition base of 0/32/64/96, so scale the
    # whole 96..127 block and DMA only partition 127's row out.
    nc.scalar.mul(G[96:128, :], O2[96:128, 0, :], 2.0)

    # Output scatter. Row r of the output plane, r = 6q + 3k + t.
    yp = ya[p]
    yp2 = yp.rearrange("(q k t) c -> q k t c", k=2, t=3)
    # out[3i+1] = F(i): rows 6q+3k+1
    nc.sync.dma_start(yp2[:, :, 1, :], O2[:, :, :])
    # out[3i+3] = F(i), i=2q   -> rows 6q+3
    nc.sync.dma_start(yp2[:, 1, 0, :], O2[:, 0, :])
    # out[3i+3] = F(i), i=2q+1 -> rows 6q+6
    nc.scalar.dma_start(yp2[1:128, 0, 0, :], O2[0:127, 1, :])
    # out[3i-1] = F(i), i=2q   -> rows 6q-1 (q>=1)
    nc.sync.dma_start(yp2[0:127, 1, 2, :], O2[1:128, 0, :])
    # out[3i-1] = F(i), i=2q+1 -> rows 6q+2
    nc.scalar.dma_start(yp2[:, 0, 2, :], O2[:, 1, :])
    # out[0] = F(1)
    nc.scalar.dma_start(yp[0:1, :], O2[0:1, 1, :])
    # out[767] = 2*F(254)
    nc.scalar.dma_start(yp[767:768, :], G[127:128, :])


def _get_nc(n_iter: int = 1):
    if n_iter not in _NC_CACHE:
        _NC_CACHE[n_iter] = _build_nc(n_iter)
    return _NC_CACHE[n_iter]


def kernel(x: np.ndarray) -> np.ndarray:
    from concourse.bass_utils import run_bass_kernel_spmd

    x = np.ascontiguousarray(x, dtype=np.float32)
    b, c, h, w = x.shape
    assert (b, c, h, w) == (4, 32, H, W), (b, c, h, w)
    planes = x.reshape(N_CORES, PLANES_PER_CORE, H, W)

    nc = _get_nc(1)
    in_maps = [{"x": planes[i]} for i in range(N_CORES)]
    res = run_bass_kernel_spmd(nc, in_maps, core_ids=list(range(N_CORES)))
    out = np.stack([res.results[i]["y"] for i in range(N_CORES)], axis=0)
    return out.reshape(b, c, 3 * H, 3 * W)
